# revision 10
# baseline (speedup 1.0000x reference)
"""Trainium2 Bass kernel for YOLO-style DetectionLayer decode.

Full input  x: (16, 255, 76, 76) f32  (channel-major: 3 anchors x 85 ch)
Full output  : (16, 17328, 85) f32   (position-major: 3*76*76 rows x 85 ch)

Math per (b, a, gy, gx):
  out[..., 0] = (sigmoid(tx) + gx) * 8
  out[..., 1] = (sigmoid(ty) + gy) * 8
  out[..., 2] = exp(tw) * ANCHOR[a][0]        (stride cancels)
  out[..., 3] = exp(th) * ANCHOR[a][1]
  out[..., 4:] = sigmoid(...)

Sharding: pure data-parallel over batch: 2 batches per core x 8 cores.

Per-core kernel (per (b, a) pair, 6 pairs):
  - 6 independent input tiles, one per pair, each loading exactly the 85
    real channel rows (minimal HBM traffic).  Tiles alternate partition
    bases 0/43 so the fixed partition->SDMA-engine map stays balanced
    across the 16 engines.  Loads ride the scalar HWDGE queue: RTL
    descriptor generation (no Q7 software loop), and with no tile reuse
    there are no WAR semaphore waits to stall the issuing engine.
  - TensorE transposes 46 chunks of (85 ch, 128 pos) -> PSUM
    (128 pos, 85 ch) with an 85x85 identity selector: the contraction
    covers only the 85 real channels, so no junk data enters the PE
    array.  Chunk j takes positions {45 p + j} so output partition p
    holds 45 consecutive output rows -> 15.3KB contiguous store runs.
    Chunks pack at 85-col offsets, 6 per PSUM bank (510 of 512 cols).
  - ScalarE evacuates each bank with one fused tanh(v/2) over the
    contiguous 510 cols (sigmoid = .5+.5*tanh), plus true Exp on the
    w/h cols straight from PSUM raw values.
  - VectorE: whole-tile affine .5*t+.5 (2x port mode) turns tanh into
    sigmoid; x/y = 8*s + 8*grid (host table); w/h = (2A)*v - A
    (compensating the affine on the exp'd cols).
  - Main stores ride the sync HWDGE queue; the six 16-position tails
    accumulate in one SBUF tile and go out in a single strided store.
"""

import os
import sys

import numpy as np

for _p in ("/opt/trn_rl_repo", "/root/.axon_site/_ro/trn_rl_repo"):
    if os.path.isdir(_p) and _p not in sys.path:
        sys.path.append(_p)

import concourse.bacc as bacc
import concourse.bass as bass
import concourse.mybir as mybir
import concourse.tile as tile
from concourse.bass_utils import run_bass_kernel_spmd

ANCHORS = np.array([[10.0, 13.0], [16.0, 30.0], [33.0, 23.0]], dtype=np.float32)
NB_FULL = 16
N_CORES = 8
NB = NB_FULL // N_CORES  # batches per core
NA = 3
NC = 85  # 5 + 80 channels
NG = 76
NPOS = NG * NG  # 5776
STRIDE = 8.0
NPAIR = NB * NA  # 6

# Position-chunking: output partition p holds rows [45p, 45p+45); chunk j
# gathers positions {45p + j}. 5776 = 128*45 + 16 -> 16-row tail.
RPP = 45  # rows per partition (main part)
MAIN = 128 * RPP  # 5760
TAIL = NPOS - MAIN  # 16

BASE_B = 128 - NC  # 43: odd pairs put channel c at partition 43+c
PAIR_BASE = [0, BASE_B, 0, BASE_B, 0, BASE_B]

# PSUM bank packing.  Even pairs: transposes write exactly 85 cols -> 6
# chunks per bank (510 of 512 cols).  Odd pairs: transpose mode needs a
# square 128x128 permutation, so each write is 128 cols at an 85-col
# stride -- chunk m's 43 junk cols land where chunk m+1's real cols go
# and are overwritten; only the last chunk's junk tail survives
# (cols 425..467 < 512), so evacuation stays contiguous: 5 per bank.
def _groups(cpb):
    return [(g * cpb, min(cpb, RPP - g * cpb)) for g in range(-(-RPP // cpb))]


GROUPS_EVEN = _groups(6)  # 7x6 + 3
GROUPS_ODD = _groups(5)  # 9x5

F32 = mybir.dt.float32
AF = mybir.ActivationFunctionType
OP = mybir.AluOpType


def _tables():
    p = np.arange(128)[:, None]
    j = np.arange(RPP)[None, :]
    r = p * RPP + j
    gg = np.empty((128, 2 * RPP), dtype=np.float32)
    gg[:, 0::2] = (r % NG) * STRIDE
    gg[:, 1::2] = (r // NG) * STRIDE
    rt = MAIN + np.arange(TAIL)
    gxt = ((rt % NG) * STRIDE).astype(np.float32)[:, None]
    gyt = float((MAIN // NG) * STRIDE)  # rows 5760..5775 all have gy=75
    assert np.all(rt // NG == MAIN // NG)
    # selectors: base-0 pairs use an 85x85 identity (contraction covers
    # only the real channels); base-43 pairs need a square 128x128
    # permutation for transpose mode: row 43+c -> col c, junk rows
    # 0..42 -> junk cols 85..127.
    ident = np.zeros((128, 128), dtype=np.float32)
    ident[BASE_B + np.arange(NC), np.arange(NC)] = 1.0
    ident[np.arange(BASE_B), NC + np.arange(BASE_B)] = 1.0
    ident0 = np.zeros((NC, NC), dtype=np.float32)
    ident0[np.arange(NC), np.arange(NC)] = 1.0
    return gg, gxt, gyt, ident0, ident


GG_TABLE, GXT_TABLE, GYT_CONST, IDENT0_TABLE, IDENT43_TABLE = _tables()


def build_program():
    nc = bacc.Bacc(None, target_bir_lowering=False)

    x = nc.dram_tensor("x", (NB, NA * NC, NG, NG), F32, kind="ExternalInput")
    out = nc.dram_tensor("out", (NB, NA * NPOS, NC), F32, kind="ExternalOutput")
    gg = nc.dram_tensor("gg", (128, 2 * RPP), F32, kind="ExternalInput")
    gxt = nc.dram_tensor("gxt", (TAIL, 1), F32, kind="ExternalInput")
    id0 = nc.dram_tensor("id0", (NC, NC), F32, kind="ExternalInput")
    id43 = nc.dram_tensor("id43", (128, 128), F32, kind="ExternalInput")

    with tile.TileContext(nc) as tc:
        with (
            tc.tile_pool(name="constp", bufs=1) as constp,
            tc.tile_pool(name="xp", bufs=1) as xp,
            tc.tile_pool(name="outp", bufs=3) as outp,
            tc.tile_pool(name="pp", bufs=4, space="PSUM") as pp,
            tc.tile_pool(name="tp", bufs=2, space="PSUM") as tp,
        ):
            # small constants on the sync queue (free at start; done in ~2us)
            id0s = constp.tile([NC, NC], F32)
            nc.sync.dma_start(out=id0s[:], in_=id0[:])
            id43s = constp.tile([128, 128], F32)
            nc.sync.dma_start(out=id43s[:], in_=id43[:])
            ggs = constp.tile([128, 2 * RPP], F32)
            nc.sync.dma_start(out=ggs[:], in_=gg[:])
            gxts = constp.tile([TAIL, 1], F32)
            nc.sync.dma_start(out=gxts[:], in_=gxt[:])
            ggv = ggs.rearrange("p (k c) -> p k c", c=2)

            xf = x.rearrange("b c h w -> (b c) (h w)")

            # one tile per pair; only the 85 real rows are DMA'd.  Odd tiles
            # (base 43) feed the PE as full 128-partition operands (engine
            # SBUF access must start at partition 0), so their 43 junk
            # partitions get a one-time memset: the selector's zero rows
            # multiply them by 0, which is only safe for finite values.
            xts = [xp.tile([128, NPOS], F32, name=f"xt{i}") for i in range(NPAIR)]
            for pair in range(NPAIR):
                base = PAIR_BASE[pair]
                nc.scalar.dma_start(
                    out=xts[pair][base : base + NC, :],
                    in_=xf[pair * NC : (pair + 1) * NC, :],
                )
            for pair in range(NPAIR):
                if PAIR_BASE[pair]:
                    nc.vector.memset(xts[pair][0:BASE_B, :], 0.0)

            # all six 16-position tails accumulate here; one store at the end
            tta = constp.tile([TAIL, 512], F32)

            for pair in range(NPAIR):
                b, a = divmod(pair, NA)
                aw = float(ANCHORS[a, 0])
                ah = float(ANCHORS[a, 1])
                base = PAIR_BASE[pair]
                # even pairs: 85-partition operands at base 0, 85-col writes;
                # odd pairs: full 128-partition operands with the square
                # permutation, 128-col writes at 85-col stride (overwrite
                # packing)
                even = base == 0
                sel = id0s[:, :] if even else id43s[:, :]
                ow = NC if even else 128  # transpose output col width
                xt = xts[pair][0 : base + NC, :]
                ot = outp.tile([128, RPP * NC + 1], F32, tag="ot")
                # (ch, 45, 128): [:, j, :] = chunk j (stride-45 positions)
                xmain = xt[:, 0:MAIN].rearrange("c (m j) -> c j m", j=RPP)

                for k0, nk in GROUPS_EVEN if even else GROUPS_ODD:
                    ps = pp.tile([128, 512], F32, tag="ps")
                    for m in range(nk):
                        nc.tensor.transpose(
                            ps[:, NC * m : NC * m + ow],
                            xmain[:, k0 + m, :],
                            sel,
                            tile_position=(0, 0),
                        )
                    # evacuate with fused tanh(v/2) over the contiguous bank,
                    # then true exp on the w/h cols from PSUM raw values
                    nc.scalar.activation(
                        ot[:, k0 * NC : (k0 + nk) * NC],
                        ps[:, 0 : nk * NC],
                        AF.Tanh,
                        scale=0.5,
                    )
                    psv = ps[:, 0 : nk * NC].rearrange("p (k c) -> p k c", c=NC)
                    otv = ot[:, k0 * NC : (k0 + nk) * NC].rearrange(
                        "p (k c) -> p k c", c=NC
                    )
                    nc.scalar.activation(otv[:, :, 2:4], psv[:, :, 2:4], AF.Exp)

                # tail: positions 5760..5775
                pst = tp.tile([TAIL, 512], F32, tag="pst")
                nc.tensor.transpose(
                    pst[:, 0:ow], xt[:, MAIN:NPOS], sel, tile_position=(0, 0)
                )
                tt = tta[:, pair * NC : (pair + 1) * NC]
                nc.scalar.activation(tt, pst[:, 0:NC], AF.Tanh, scale=0.5)
                nc.scalar.activation(tt[:, 2:4], pst[:, 2:4], AF.Exp)

                # VectorE fixups (main): whole-tile affine at 2x port mode
                # (needs an even element count -> one memset pad column),
                # then per-channel-type corrections.
                nc.vector.memset(ot[:, RPP * NC : RPP * NC + 1], 0.0)
                nc.vector.tensor_scalar(
                    ot[:, 0 : RPP * NC + 1],
                    ot[:, 0 : RPP * NC + 1],
                    0.5,
                    0.5,
                    OP.mult,
                    OP.add,
                )
                otr = ot[:, 0 : RPP * NC].rearrange("p (k c) -> p k c", c=NC)
                xy = otr[:, :, 0:2]
                nc.vector.tensor_scalar(xy, xy, STRIDE, None, OP.mult)
                nc.vector.tensor_tensor(xy, xy, ggv, OP.add)
                wv = otr[:, :, 2:3]
                nc.vector.tensor_scalar(wv, wv, 2.0 * aw, -aw, OP.mult, OP.add)
                hv = otr[:, :, 3:4]
                nc.vector.tensor_scalar(hv, hv, 2.0 * ah, -ah, OP.mult, OP.add)

                # VectorE fixups (tail); odd count (85) -> affine over 84
                # then the last col separately
                nc.vector.tensor_scalar(
                    tt[:, 0:84], tt[:, 0:84], 0.5, 0.5, OP.mult, OP.add
                )
                nc.vector.tensor_scalar(
                    tt[:, 84:85], tt[:, 84:85], 0.5, 0.5, OP.mult, OP.add
                )
                nc.vector.tensor_scalar(
                    tt[:, 0:1], tt[:, 0:1], STRIDE, gxts[:], OP.mult, OP.add
                )
                nc.vector.tensor_scalar(
                    tt[:, 1:2], tt[:, 1:2], STRIDE, GYT_CONST, OP.mult, OP.add
                )
                nc.vector.tensor_scalar(
                    tt[:, 2:3], tt[:, 2:3], 2.0 * aw, -aw, OP.mult, OP.add
                )
                nc.vector.tensor_scalar(
                    tt[:, 3:4], tt[:, 3:4], 2.0 * ah, -ah, OP.mult, OP.add
                )

                # main store on the sync HWDGE queue: 128 runs of 15.3KB
                obase = a * NPOS
                nc.sync.dma_start(
                    out=out[b, obase : obase + MAIN, :].rearrange(
                        "(p j) c -> p (j c)", p=128
                    ),
                    in_=ot[:, 0 : RPP * NC],
                )

            # one combined tail store: out[b, a*NPOS + 5760 + t, c] with
            # partition t and free (b, a, c) = tta col (b*3+a)*85 + c
            tails = out.rearrange("b (a q) c -> q b a c", a=NA)
            nc.sync.dma_start(
                out=tails[MAIN:NPOS],
                in_=tta[:, 0 : NPAIR * NC].rearrange(
                    "t (b a c) -> t b a c", b=NB, a=NA
                ),
            )

    nc.compile()
    return nc


_NC_CACHE = None


def _get_program():
    global _NC_CACHE
    if _NC_CACHE is None:
        _NC_CACHE = build_program()
    return _NC_CACHE


def run(x, trace=False, **kwargs):
    """x: full (16, 255, 76, 76) f32. Returns (full_out, BassKernelResults)."""
    x = np.ascontiguousarray(np.asarray(x, dtype=np.float32))
    assert x.shape == (NB_FULL, NA * NC, NG, NG), x.shape
    nc = _get_program()
    in_maps = [
        {
            "x": np.ascontiguousarray(x[c * NB : (c + 1) * NB]),
            "gg": GG_TABLE,
            "gxt": GXT_TABLE,
            "id0": IDENT0_TABLE,
            "id43": IDENT43_TABLE,
        }
        for c in range(N_CORES)
    ]
    res = run_bass_kernel_spmd(nc, in_maps, list(range(N_CORES)), trace=trace, **kwargs)
    out = np.concatenate([res.results[c]["out"] for c in range(N_CORES)], axis=0)
    return out, res


def kernel(x):
    out, _ = run(x, trace=False)
    return out


# revision 11
# speedup vs baseline: 1.5735x; 1.5735x over previous
"""Trainium2 Bass kernel for YOLO-style DetectionLayer decode.

Full input  x: (16, 255, 76, 76) f32  (channel-major: 3 anchors x 85 ch)
Full output  : (16, 17328, 85) f32   (position-major: 3*76*76 rows x 85 ch)

Math per (b, a, gy, gx):
  out[..., 0] = (sigmoid(tx) + gx) * 8
  out[..., 1] = (sigmoid(ty) + gy) * 8
  out[..., 2] = exp(tw) * ANCHOR[a][0]        (stride cancels)
  out[..., 3] = exp(th) * ANCHOR[a][1]
  out[..., 4:] = sigmoid(...)

Sharding: pure data-parallel over batch: 2 batches per core x 8 cores.

Per-core kernel (per (b, a) pair, 6 pairs):
  - 6 independent input tiles, one per pair, each loading exactly the 85
    real channel rows (minimal HBM traffic).  Tiles alternate partition
    bases 0/43 so the fixed partition->SDMA-engine map stays balanced
    across the 16 engines.  Loads ride the scalar HWDGE queue: RTL
    descriptor generation (no Q7 software loop), and with no tile reuse
    there are no WAR semaphore waits to stall the issuing engine.
  - TensorE transposes 46 chunks of (85 ch, 128 pos) -> PSUM
    (128 pos, 85 ch) with an 85x85 identity selector: the contraction
    covers only the 85 real channels, so no junk data enters the PE
    array.  Chunk j takes positions {45 p + j} so output partition p
    holds 45 consecutive output rows -> 15.3KB contiguous store runs.
    Chunks pack at 85-col offsets, 6 per PSUM bank (510 of 512 cols).
  - ScalarE evacuates each bank with one fused tanh(v/2) over the
    contiguous 510 cols (sigmoid = .5+.5*tanh), plus true Exp on the
    w/h cols straight from PSUM raw values.
  - VectorE: whole-tile affine .5*t+.5 (2x port mode) turns tanh into
    sigmoid; x/y = 8*s + 8*grid (host table); w/h = (2A)*v - A
    (compensating the affine on the exp'd cols).
  - Main stores ride the sync HWDGE queue; the six 16-position tails
    accumulate in one SBUF tile and go out in a single strided store.
"""

import os
import sys

import numpy as np

for _p in ("/opt/trn_rl_repo", "/root/.axon_site/_ro/trn_rl_repo"):
    if os.path.isdir(_p) and _p not in sys.path:
        sys.path.append(_p)

import concourse.bacc as bacc
import concourse.bass as bass
import concourse.mybir as mybir
import concourse.tile as tile
from concourse.bass_utils import run_bass_kernel_spmd

ANCHORS = np.array([[10.0, 13.0], [16.0, 30.0], [33.0, 23.0]], dtype=np.float32)
NB_FULL = 16
N_CORES = 8
NB = NB_FULL // N_CORES  # batches per core
NA = 3
NC = 85  # 5 + 80 channels
NG = 76
NPOS = NG * NG  # 5776
STRIDE = 8.0
NPAIR = NB * NA  # 6

# Position-chunking: output partition p holds rows [45p, 45p+45); chunk j
# gathers positions {45p + j}. 5776 = 128*45 + 16 -> 16-row tail.
RPP = 45  # rows per partition (main part)
MAIN = 128 * RPP  # 5760
TAIL = NPOS - MAIN  # 16

BASE_B = 128 - NC  # 43: odd pairs put channel c at partition 43+c
PAIR_BASE = [0, BASE_B, 0, BASE_B, 0, BASE_B]

# PSUM bank packing.  Even pairs: transposes write exactly 85 cols -> 6
# chunks per bank (510 of 512 cols).  Odd pairs: transpose mode needs a
# square 128x128 permutation, so each write is 128 cols at an 85-col
# stride -- chunk m's 43 junk cols land where chunk m+1's real cols go
# and are overwritten; only the last chunk's junk tail survives
# (cols 425..467 < 512), so evacuation stays contiguous: 5 per bank.
def _groups(cpb):
    return [(g * cpb, min(cpb, RPP - g * cpb)) for g in range(-(-RPP // cpb))]


GROUPS_EVEN = _groups(6)  # 7x6 + 3
GROUPS_ODD = _groups(5)  # 9x5

F32 = mybir.dt.float32
AF = mybir.ActivationFunctionType
OP = mybir.AluOpType


def _tables():
    p = np.arange(128)[:, None]
    j = np.arange(RPP)[None, :]
    r = p * RPP + j
    gg = np.empty((128, 2 * RPP), dtype=np.float32)
    gg[:, 0::2] = (r % NG) * STRIDE
    gg[:, 1::2] = (r // NG) * STRIDE
    rt = MAIN + np.arange(TAIL)
    gxt = ((rt % NG) * STRIDE).astype(np.float32)[:, None]
    gyt = float((MAIN // NG) * STRIDE)  # rows 5760..5775 all have gy=75
    assert np.all(rt // NG == MAIN // NG)
    # selectors: base-0 pairs use an 85x85 identity (contraction covers
    # only the real channels); base-43 pairs need a square 128x128
    # permutation for transpose mode: row 43+c -> col c, junk rows
    # 0..42 -> junk cols 85..127.
    ident = np.zeros((128, 128), dtype=np.float32)
    ident[BASE_B + np.arange(NC), np.arange(NC)] = 1.0
    ident[np.arange(BASE_B), NC + np.arange(BASE_B)] = 1.0
    ident0 = np.zeros((NC, NC), dtype=np.float32)
    ident0[np.arange(NC), np.arange(NC)] = 1.0
    return gg, gxt, gyt, ident0, ident


GG_TABLE, GXT_TABLE, GYT_CONST, IDENT0_TABLE, IDENT43_TABLE = _tables()


def build_program():
    nc = bacc.Bacc(None, target_bir_lowering=False)

    x = nc.dram_tensor("x", (NB, NA * NC, NG, NG), F32, kind="ExternalInput")
    out = nc.dram_tensor("out", (NB, NA * NPOS, NC), F32, kind="ExternalOutput")
    gg = nc.dram_tensor("gg", (128, 2 * RPP), F32, kind="ExternalInput")
    gxt = nc.dram_tensor("gxt", (TAIL, 1), F32, kind="ExternalInput")
    id0 = nc.dram_tensor("id0", (NC, NC), F32, kind="ExternalInput")
    id43 = nc.dram_tensor("id43", (128, 128), F32, kind="ExternalInput")

    with tile.TileContext(nc) as tc:
        with (
            tc.tile_pool(name="constp", bufs=1) as constp,
            tc.tile_pool(name="xp", bufs=1) as xp,
            tc.tile_pool(name="outp", bufs=3) as outp,
            tc.tile_pool(name="pp", bufs=4, space="PSUM") as pp,
            tc.tile_pool(name="tp", bufs=2, space="PSUM") as tp,
        ):
            # small constants on the sync queue (free at start; done in ~2us)
            id0s = constp.tile([NC, NC], F32)
            nc.sync.dma_start(out=id0s[:], in_=id0[:])
            id43s = constp.tile([128, 128], F32)
            nc.sync.dma_start(out=id43s[:], in_=id43[:])
            ggs = constp.tile([128, 2 * RPP], F32)
            nc.sync.dma_start(out=ggs[:], in_=gg[:])
            gxts = constp.tile([TAIL, 1], F32)
            nc.sync.dma_start(out=gxts[:], in_=gxt[:])
            ggv = ggs.rearrange("p (k c) -> p k c", c=2)

            xf = x.rearrange("b c h w -> (b c) (h w)")

            # one tile per pair; only the 85 real rows are DMA'd, on the
            # gpsimd SWDGE queue (the Q7 CounterMachine generates packet
            # descriptors ~5x faster than the HWDGE RTL, and with no tile
            # reuse these dma_starts carry no semaphore waits that could
            # stall the in-order issue stream).  Odd tiles (base 43) feed
            # the PE as full 128-partition operands; their 43 junk
            # partitions stay uninitialized -- transpose mode is pure
            # routing, so junk rows land only in junk columns.
            xts = [xp.tile([128, NPOS], F32, name=f"xt{i}") for i in range(NPAIR)]
            for pair in range(NPAIR):
                base = PAIR_BASE[pair]
                nc.gpsimd.dma_start(
                    out=xts[pair][base : base + NC, :],
                    in_=xf[pair * NC : (pair + 1) * NC, :],
                )

            # all six 16-position tails accumulate here; one store at the end
            tta = constp.tile([TAIL, 512], F32)

            for pair in range(NPAIR):
                b, a = divmod(pair, NA)
                aw = float(ANCHORS[a, 0])
                ah = float(ANCHORS[a, 1])
                base = PAIR_BASE[pair]
                # even pairs: 85-partition operands at base 0, 85-col writes;
                # odd pairs: full 128-partition operands with the square
                # permutation, 128-col writes at 85-col stride (overwrite
                # packing)
                even = base == 0
                sel = id0s[:, :] if even else id43s[:, :]
                ow = NC if even else 128  # transpose output col width
                xt = xts[pair][0 : base + NC, :]
                ot = outp.tile([128, RPP * NC + 1], F32, tag="ot")
                # (ch, 45, 128): [:, j, :] = chunk j (stride-45 positions)
                xmain = xt[:, 0:MAIN].rearrange("c (m j) -> c j m", j=RPP)

                for k0, nk in GROUPS_EVEN if even else GROUPS_ODD:
                    ps = pp.tile([128, 512], F32, tag="ps")
                    for m in range(nk):
                        nc.tensor.transpose(
                            ps[:, NC * m : NC * m + ow],
                            xmain[:, k0 + m, :],
                            sel,
                            tile_position=(0, 0),
                        )
                    # evacuate with fused tanh(v/2) over the contiguous bank,
                    # then true exp on the w/h cols from PSUM raw values
                    nc.scalar.activation(
                        ot[:, k0 * NC : (k0 + nk) * NC],
                        ps[:, 0 : nk * NC],
                        AF.Tanh,
                        scale=0.5,
                    )
                    psv = ps[:, 0 : nk * NC].rearrange("p (k c) -> p k c", c=NC)
                    otv = ot[:, k0 * NC : (k0 + nk) * NC].rearrange(
                        "p (k c) -> p k c", c=NC
                    )
                    nc.scalar.activation(otv[:, :, 2:4], psv[:, :, 2:4], AF.Exp)

                # tail: positions 5760..5775
                pst = tp.tile([TAIL, 512], F32, tag="pst")
                nc.tensor.transpose(
                    pst[:, 0:ow], xt[:, MAIN:NPOS], sel, tile_position=(0, 0)
                )
                tt = tta[:, pair * NC : (pair + 1) * NC]
                nc.scalar.activation(tt, pst[:, 0:NC], AF.Tanh, scale=0.5)
                nc.scalar.activation(tt[:, 2:4], pst[:, 2:4], AF.Exp)

                # VectorE fixups (main): whole-tile affine at 2x port mode
                # (needs an even element count -> one memset pad column),
                # then per-channel-type corrections.
                nc.vector.memset(ot[:, RPP * NC : RPP * NC + 1], 0.0)
                nc.vector.tensor_scalar(
                    ot[:, 0 : RPP * NC + 1],
                    ot[:, 0 : RPP * NC + 1],
                    0.5,
                    0.5,
                    OP.mult,
                    OP.add,
                )
                otr = ot[:, 0 : RPP * NC].rearrange("p (k c) -> p k c", c=NC)
                xy = otr[:, :, 0:2]
                nc.vector.tensor_scalar(xy, xy, STRIDE, None, OP.mult)
                nc.vector.tensor_tensor(xy, xy, ggv, OP.add)
                wv = otr[:, :, 2:3]
                nc.vector.tensor_scalar(wv, wv, 2.0 * aw, -aw, OP.mult, OP.add)
                hv = otr[:, :, 3:4]
                nc.vector.tensor_scalar(hv, hv, 2.0 * ah, -ah, OP.mult, OP.add)

                # VectorE fixups (tail); odd count (85) -> affine over 84
                # then the last col separately
                nc.vector.tensor_scalar(
                    tt[:, 0:84], tt[:, 0:84], 0.5, 0.5, OP.mult, OP.add
                )
                nc.vector.tensor_scalar(
                    tt[:, 84:85], tt[:, 84:85], 0.5, 0.5, OP.mult, OP.add
                )
                nc.vector.tensor_scalar(
                    tt[:, 0:1], tt[:, 0:1], STRIDE, gxts[:], OP.mult, OP.add
                )
                nc.vector.tensor_scalar(
                    tt[:, 1:2], tt[:, 1:2], STRIDE, GYT_CONST, OP.mult, OP.add
                )
                nc.vector.tensor_scalar(
                    tt[:, 2:3], tt[:, 2:3], 2.0 * aw, -aw, OP.mult, OP.add
                )
                nc.vector.tensor_scalar(
                    tt[:, 3:4], tt[:, 3:4], 2.0 * ah, -ah, OP.mult, OP.add
                )

                # main store on the sync HWDGE queue: 128 runs of 15.3KB
                obase = a * NPOS
                nc.sync.dma_start(
                    out=out[b, obase : obase + MAIN, :].rearrange(
                        "(p j) c -> p (j c)", p=128
                    ),
                    in_=ot[:, 0 : RPP * NC],
                )

            # one combined tail store: out[b, a*NPOS + 5760 + t, c] with
            # partition t and free (b, a, c) = tta col (b*3+a)*85 + c
            tails = out.rearrange("b (a q) c -> q b a c", a=NA)
            nc.sync.dma_start(
                out=tails[MAIN:NPOS],
                in_=tta[:, 0 : NPAIR * NC].rearrange(
                    "t (b a c) -> t b a c", b=NB, a=NA
                ),
            )

    nc.compile()
    return nc


_NC_CACHE = None


def _get_program():
    global _NC_CACHE
    if _NC_CACHE is None:
        _NC_CACHE = build_program()
    return _NC_CACHE


def run(x, trace=False, **kwargs):
    """x: full (16, 255, 76, 76) f32. Returns (full_out, BassKernelResults)."""
    x = np.ascontiguousarray(np.asarray(x, dtype=np.float32))
    assert x.shape == (NB_FULL, NA * NC, NG, NG), x.shape
    nc = _get_program()
    in_maps = [
        {
            "x": np.ascontiguousarray(x[c * NB : (c + 1) * NB]),
            "gg": GG_TABLE,
            "gxt": GXT_TABLE,
            "id0": IDENT0_TABLE,
            "id43": IDENT43_TABLE,
        }
        for c in range(N_CORES)
    ]
    res = run_bass_kernel_spmd(nc, in_maps, list(range(N_CORES)), trace=trace, **kwargs)
    out = np.concatenate([res.results[c]["out"] for c in range(N_CORES)], axis=0)
    return out, res


def kernel(x):
    out, _ = run(x, trace=False)
    return out
